# revision 36
# baseline (speedup 1.0000x reference)
"""Trainium2 Bass kernel for the ConditioningEncoder GNN message-passing model.

Math restructuring (the key to the memory-regime roofline): the reference
materializes (k,n,n,H) message tensors, but with
  edge_fts[i,j,:] = A[i,j]*We0 + adj[i,j]*We1 + pred[i,j]*We2 + be
the masked aggregation decomposes into
  msg_agg[j,:] = ( sum_i adj_self[i,j]*nf[i,:]            # one (N,N)@(N,H) matmul
                 + cA[j]*We0 + cadj[j]*We1 + cpred[j]*We2 + cdeg[j]*be ) / deg[j]
where cA/cadj/cpred/cdeg are per-node column reductions over adj_self computed
ONCE (shared by both MP rounds).  Everything is kept feature-major
(H on partitions, nodes on the free axis) so each round is 4 PSUM-accumulated
matmuls + two 64x512 MLP matmuls.

The heavy matmul streams run in bf16 (fp32 matmul is 4 cycles/row on the PE;
bf16 is 1).  adj values {0,1,2} are exact in bf16; accumulation stays fp32 in
PSUM.  The pi (one-hot index) compare path stays fp32 for integer exactness.

Sharding: data-parallel over k (16 examples / 8 cores = 2 per core), params
replicated, on-device AllReduce of the (1,128) partial mean at the end.
"""

import sys

sys.path.insert(0, "/opt/trn_rl_repo")

import numpy as np

import concourse.bass as bass
import concourse.bacc as bacc
import concourse.mybir as mybir
from concourse import tile
from concourse.bass_utils import run_bass_kernel_spmd

K, N, T, H, Z = 16, 512, 8, 64, 128
NCORES = 8
KLOC = K // NCORES  # 2 examples per core
P = 128             # SBUF partitions
NT = N // P         # 4 row-tiles per (N,N) matrix
F32 = mybir.dt.float32
I32 = mybir.dt.int32
BF = mybir.dt.bfloat16
AF = mybir.ActivationFunctionType
OP = mybir.AluOpType


def _encoder(tc: "tile.TileContext", io: dict, collective: bool = True,
             reps: int = 1):
    nc = tc.nc
    with (
        tc.tile_pool(name="const", bufs=1) as cpool,
        tc.tile_pool(name="big", bufs=2) as bigpool,
        tc.tile_pool(name="stage", bufs=3) as stpool,
        tc.tile_pool(name="small", bufs=2) as smpool,
        tc.tile_pool(name="psum", bufs=1, space="PSUM") as ppool,
        tc.tile_pool(name="dram", bufs=1, space="DRAM") as dpool,
    ):
        # ---- constants -------------------------------------------------
        ones_col = cpool.tile([P, 1], BF)
        nc.vector.memset(ones_col[:], 1.0)
        ident = cpool.tile([P, P], BF)
        nc.vector.memset(ident[:], 1.0)
        nc.gpsimd.affine_select(
            ident[:], ident[:], pattern=[[1, P]], compare_op=OP.is_equal,
            fill=0.0, base=0, channel_multiplier=-1,
        )
        iota_i = cpool.tile([P, NT], I32)
        nc.gpsimd.iota(iota_i[:], pattern=[[P, NT]], base=0, channel_multiplier=1)
        iota_f = cpool.tile([P, NT], F32)
        nc.vector.tensor_copy(iota_f[:], iota_i[:])

        def row(ap):  # (X,) dram AP -> (1,X)
            return ap.rearrange("(p j) -> p j", p=1)

        def col(ap):  # (X,) dram AP -> (X,1)
            return ap.rearrange("(p j) -> p j", j=1)

        # All 64-partition weights arrive host-packed in ONE tensor "wbig"
        # (64, 387): [Wmp0a|Wmp0b|Wmp1a|Wmp1b|Wz|bn|bmp0|bmp1]; one DMA, one
        # bf16 cast.  Row-vector params in "vrow" (1,448):
        # [Wn3|We0|We1|We2|be|bz].  (A per-tensor gpsimd casting DMA costs
        # ~1us of SWDGE fixed overhead each; HWDGE smalls ~0.6us each.)
        wbigF = cpool.tile([H, 387], F32)
        nc.sync.dma_start(wbigF[:], io["wbig"][:, :])
        wbig = cpool.tile([H, 384], BF)
        nc.scalar.copy(wbig[:], wbigF[:, 0:384])
        Wmp_a = [wbig[:, 0:H], wbig[:, 2 * H:3 * H]]
        Wmp_b = [wbig[:, H:2 * H], wbig[:, 3 * H:4 * H]]
        Wz_sb = wbig[:, 4 * H:4 * H + Z]
        bn_sb = wbigF[:, 384:385]
        bmp_sb = [wbigF[:, 385:386], wbigF[:, 386:387]]
        vrowF = cpool.tile([1, 448], F32)
        nc.sync.dma_start(vrowF[:], io["vrow"][:, :])
        vrow = cpool.tile([1, 320], BF)
        nc.scalar.copy(vrow[:], vrowF[:, 0:320])
        Wn4_sb = vrow[:, 0:H]
        WeR = [vrow[:, H:2 * H], vrow[:, 2 * H:3 * H], vrow[:, 3 * H:4 * H],
               vrow[:, 4 * H:5 * H]]
        bz_sb = vrowF[:, 320:448]
        Wn3F = cpool.tile([3, H], F32)
        nc.sync.dma_start(Wn3F[:], io["Wn"][0:3, :])
        Wn3_sb = cpool.tile([3, H], BF)
        nc.scalar.copy(Wn3_sb[:], Wn3F[:])

        import contextlib
        loop_ctx = (tc.For_i(0, reps, 1) if reps > 1
                    else contextlib.nullcontext())
        with loop_ctx:
          zacc = None
          for k in range(KLOC):
            if k == 0:
                # SBUF accumulator for the per-example readout partials
                zacc = smpool.tile([1, Z], F32, tag="zacc",
                                   bufs=(1 if reps == 1 else 2))
            # ---- bulk loads: adj (-> adj_self, kept) and A (streamed) --
            # src[p, t, j] = adj[k, t*P + p, j]  (row-tile t laid side by side)
            adjS = bigpool.tile([P, NT * N], BF, tag="adjS", bufs=2)
            nc.gpsimd.dma_start(
                adjS[:], bass.AP(io["adj2"], k * N * N, [[N, P], [P * N, NT], [1, N]]))
            for t in range(NT):
                d0c = t * N + t * P  # diagonal block columns of row-tile t
                nc.vector.tensor_add(adjS[:, d0c:d0c + P], adjS[:, d0c:d0c + P], ident[:])
            At = bigpool.tile([P, NT * N], BF, tag="At", bufs=2)
            nc.gpsimd.dma_start(
                At[:], bass.AP(io["A2"], k * N * N, [[N, P], [P * N, NT], [1, N]]))

            # pi_final broadcast to all partitions (Pool engine)
            pi_i = smpool.tile([1, N], I32, tag="pii")
            nc.sync.dma_start(pi_i[:], row(io["piT"][k]))
            pi_f = smpool.tile([1, N], F32, tag="pif")
            nc.vector.tensor_copy(pi_f[:], pi_i[:])
            pi_b = smpool.tile([P, N], F32, tag="pib", bufs=2)
            nc.gpsimd.partition_broadcast(pi_b[:], pi_f[:], channels=P)

            # ---- column reductions over i: cdeg, cA, cpred -------------
            rdeg = ppool.tile([1, N], F32, tag="red", bufs=3)
            rA = ppool.tile([1, N], F32, tag="red", bufs=3)
            rP = ppool.tile([1, N], F32, tag="red", bufs=3)
            for t in range(NT):
                sl = slice(t * N, (t + 1) * N)
                st = stpool.tile([P, 2 * N], BF, tag="st", bufs=3)
                # prodA = adj_self * A
                nc.vector.tensor_tensor(st[:, 0:N], adjS[:, sl], At[:, sl], op=OP.mult)
                # predprod = (pi == global_row) * adj_self   (fused DVE op)
                nc.vector.scalar_tensor_tensor(
                    st[:, N:2 * N], pi_b[:], iota_f[:, t:t + 1], adjS[:, sl],
                    op0=OP.is_equal, op1=OP.mult,
                )
                nc.tensor.matmul(rdeg[:], ones_col[:], adjS[:, sl],
                                 start=(t == 0), stop=(t == NT - 1))
                nc.tensor.matmul(rA[:], ones_col[:], st[:, 0:N],
                                 start=(t == 0), stop=(t == NT - 1))
                nc.tensor.matmul(rP[:], ones_col[:], st[:, N:2 * N],
                                 start=(t == 0), stop=(t == NT - 1))

            # coef rows: [cA; cadj; cpred; cdeg];  cadj = cdeg - 1 + diag(adj)
            cAr = smpool.tile([1, N], BF, tag="cAr")
            nc.scalar.copy(cAr[:], rA[:])
            cPr = smpool.tile([1, N], BF, tag="cPr")
            nc.scalar.copy(cPr[:], rP[:])
            cDr = smpool.tile([1, N], BF, tag="cDr")
            nc.scalar.copy(cDr[:], rdeg[:])
            diag = smpool.tile([1, N], F32, tag="diag")
            nc.sync.dma_start(diag[:], bass.AP(io["adj2"], k * N * N, [[0, 1], [N + 1, N]]))
            cJr = smpool.tile([1, N], BF, tag="cJr")
            nc.vector.scalar_tensor_tensor(cJr[:], diag[:], -1.0, rdeg[:],
                                           op0=OP.add, op1=OP.add)
            invd = smpool.tile([1, N], F32, tag="invd")
            nc.vector.reciprocal(invd[:], rdeg[:])
            invb = smpool.tile([H, N], F32, tag="invb", bufs=2)
            nc.gpsimd.partition_broadcast(invb[:], invd[:], channels=H)

            # ---- initial node features (feature-major) -----------------
            # host-packed sdd[k] = [s; d0; dT] rows; ddp[k] = [d0 | dT]
            rawF = smpool.tile([3, N], F32, tag="rawF")
            nc.sync.dma_start(rawF[:], io["sdd"][k])
            rawT = smpool.tile([3, N], BF, tag="rawT")
            nc.scalar.copy(rawT[:], rawF[:])
            dd = smpool.tile([1, 2 * N], F32, tag="dd")
            nc.sync.dma_start(dd[:], row(io["ddp"][k]))
            delta = smpool.tile([1, N], BF, tag="delta")
            nc.vector.tensor_sub(delta[:], dd[:, N:2 * N], dd[:, 0:N])
            nf0_ps = ppool.tile([H, N], F32, tag="bigmm", bufs=5)
            nc.tensor.matmul(nf0_ps[:], Wn3_sb[:], rawT[:], start=True, stop=False)
            nc.tensor.matmul(nf0_ps[:], Wn4_sb, delta[:], start=False, stop=True)
            nfT = smpool.tile([H, N], BF, tag="nfT", bufs=3, name="nfT0")
            nc.scalar.activation(nfT[:], nf0_ps[:], AF.Identity, bias=bn_sb)

            def transpose_to_nat(srcT):
                # (H, N) feature-major -> (P, NT*H) natural node-major chunks
                nfN = smpool.tile([P, NT * H], BF, tag="nfN", bufs=2,
                                  name=f"nfN_{k}")
                for t in range(NT):
                    tp = ppool.tile([P, H], BF, tag="bigmm", bufs=5)
                    nc.tensor.transpose(tp[:], srcT[:, t * P:(t + 1) * P],
                                        ident[0:H, 0:H])
                    nc.scalar.copy(nfN[:, t * H:(t + 1) * H], tp[:])
                return nfN

            nfN = transpose_to_nat(nfT[:])

            # ---- two message-passing rounds ----------------------------
            nfT2 = None
            for r in range(2):
                # ST accumulates the adjacency matmuls AND the four rank-1
                # coef outer products (edge-feature contribution) in PSUM
                ST = ppool.tile([H, N], F32, tag="bigmm", bufs=5)
                for t in range(NT):
                    nc.tensor.matmul(ST[:], nfN[:, t * H:(t + 1) * H],
                                     adjS[:, t * N:(t + 1) * N],
                                     start=(t == 0), stop=False)
                for c, cr in enumerate([cAr, cJr, cPr, cDr]):
                    nc.tensor.matmul(ST[:], WeR[c], cr[:],
                                     start=False, stop=(c == 3))
                msgT = smpool.tile([H, N], BF, tag="msgT", bufs=2)
                nc.vector.tensor_tensor(msgT[:], ST[:], invb[:], op=OP.mult)
                # MLP: Wmp[0:H]^T @ nfT + Wmp[H:2H]^T @ msgT (PSUM accumulated)
                nfx = ppool.tile([H, N], F32, tag="bigmm", bufs=5)
                nc.tensor.matmul(nfx[:], Wmp_a[r], nfT[:],
                                 start=True, stop=False)
                nc.tensor.matmul(nfx[:], Wmp_b[r], msgT[:],
                                 start=False, stop=True)
                if r == 0:
                    nfT = smpool.tile([H, N], BF, tag="nfT", bufs=3, name="nfT1")
                    nc.scalar.activation(nfT[:], nfx[:], AF.Relu, bias=bmp_sb[r])
                    nfN = transpose_to_nat(nfT[:])
                else:
                    nfT2 = smpool.tile([H, N], BF, tag="nfT", bufs=3, name="nfT2")
                    nc.scalar.activation(nfT2[:], nfx[:], AF.Relu, bias=bmp_sb[r])

            # ---- readout: (mean_j nf2) @ Wz, folded 1/(N*K) ------------
            mrow = smpool.tile([H, 1], F32, tag="mrow")
            nc.vector.tensor_reduce(mrow[:], nfT2[:], axis=mybir.AxisListType.X,
                                    op=OP.add)
            mrow2 = smpool.tile([H, 1], BF, tag="mrow2")
            nc.scalar.mul(mrow2[:], mrow[:], 1.0 / (N * K))
            ez = ppool.tile([1, Z], F32, tag="red", bufs=3)
            nc.tensor.matmul(ez[:], mrow2[:], Wz_sb, start=True, stop=True)
            if k == 0:
                nc.scalar.copy(zacc[:], ez[:])
            else:
                nc.vector.tensor_add(zacc[:], zacc[:], ez[:])

        # ---- all-reduce partial means across cores, add bz -------------
        zp = zacc
        cc_in = dpool.tile([1, Z], F32, tag="ccin")
        cc_out = dpool.tile([1, Z], F32, tag="ccout",
                            addr_space="Shared" if collective else "Local")
        nc.sync.dma_start(cc_in[:], zp[:])
        if collective:
            nc.gpsimd.collective_compute(
                "AllReduce", OP.add, replica_groups=[list(range(NCORES))],
                ins=[cc_in.opt()], outs=[cc_out.opt()],
            )
        else:
            nc.gpsimd.dma_start(cc_out[:], cc_in[:])
        zs = smpool.tile([1, Z], F32, tag="zs")
        nc.sync.dma_start(zs[:], cc_out[:])
        zf = smpool.tile([1, Z], F32, tag="zf")
        nc.vector.tensor_add(zf[:], zs[:], bz_sb)
        nc.sync.dma_start(io["z"][:].rearrange("(p j) -> p j", p=1), zf[:])


def build_program(collective: bool = True, reps: int = 1) -> bass.Bass:
    nc = bacc.Bacc("TRN2", target_bir_lowering=False, num_devices=NCORES)
    io = {}
    for name, shape, dt in [
        ("A2", [KLOC, N, N], F32), ("adj2", [KLOC, N, N], F32),
        ("sdd", [KLOC, 3, N], F32), ("ddp", [KLOC, 2 * N], F32),
        ("piT", [KLOC, N], I32), ("Wn", [4, H], F32),
        ("wbig", [H, 387], F32), ("vrow", [1, 448], F32),
    ]:
        io[name] = nc.dram_tensor(name, shape, dt, kind="ExternalInput")
    io["z"] = nc.dram_tensor("z", [Z], F32, kind="ExternalOutput")
    with tile.TileContext(nc) as tc:
        _encoder(tc, io, collective=collective, reps=reps)
    nc.compile()
    return nc


_PROGRAM = None


def _get_program():
    global _PROGRAM
    if _PROGRAM is None:
        _PROGRAM = build_program()
    return _PROGRAM


def make_in_maps(s, A, adj, d_hints, pi_hints, Wn, bn, We, be,
                 Wmp0, bmp0, Wmp1, bmp1, Wz, bz):
    f32 = lambda x: np.ascontiguousarray(x, np.float32)
    # host-side packing is layout-only (concatenation of replicated params
    # and per-example row slices)
    wbig = np.concatenate(
        [f32(Wmp0[:H]), f32(Wmp0[H:]), f32(Wmp1[:H]), f32(Wmp1[H:]),
         f32(Wz), f32(bn)[:, None], f32(bmp0)[:, None], f32(bmp1)[:, None]],
        axis=1)
    vrow = np.concatenate(
        [f32(Wn[3]), f32(We[0]), f32(We[1]), f32(We[2]), f32(be),
         f32(bz)])[None, :]
    d0, dT = d_hints[0], d_hints[-1]
    sdd = np.stack([f32(s), f32(d0), f32(dT)], axis=1)          # (K,3,N)
    ddp = np.concatenate([f32(d0), f32(dT)], axis=1)            # (K,2N)
    params = dict(Wn=f32(Wn), wbig=f32(wbig), vrow=f32(vrow))
    in_maps = []
    for c in range(NCORES):
        ks = slice(c * KLOC, (c + 1) * KLOC)
        in_maps.append(dict(
            A2=f32(A[ks]),
            adj2=f32(adj[ks]),
            sdd=np.ascontiguousarray(sdd[ks]),
            ddp=np.ascontiguousarray(ddp[ks]),
            piT=np.ascontiguousarray(pi_hints[-1, ks], np.int32),
            **params,
        ))
    return in_maps


def kernel(s, A, adj, d_hints, pi_hints, Wn, bn, We, be,
           Wmp0, bmp0, Wmp1, bmp1, Wz, bz, **run_kwargs):
    args = [np.asarray(x) for x in (s, A, adj, d_hints, pi_hints, Wn, bn,
                                    We, be, Wmp0, bmp0, Wmp1, bmp1, Wz, bz)]
    nc = _get_program()
    in_maps = make_in_maps(*args)
    res = run_bass_kernel_spmd(nc, in_maps, list(range(NCORES)), **run_kwargs)
    out = np.asarray(res.results[0]["z"], np.float32).reshape(Z)
    if run_kwargs:
        return out, res
    return out


if __name__ == "__main__":
    build_program()
    print("program built OK")
